# revision 25
# baseline (speedup 1.0000x reference)
"""Trainium2 Bass kernel for a 2-layer GAT (nn_GAT_82901458747986).

Strategy (8-core SPMD, 3 launches, zero per-edge device gathers, no
DRAM roundtrip for the aggregation table):
  - Host: add self-loops, sort edges by dst, pack whole dst-groups into
    superchunks of <=640 edge slots (5 subchunks of 128) and <=128
    groups, keeping each 128-slot subchunk's rank span <= W=40; assign
    contiguous superchunk ranges to cores.  Per-edge tables are
    host-built by pure permutation (the "halo exchange").
  - L0: per-node attention logits asad[n,16] = x @ [W1@a_src | W1@a_dst]
    via one streamed matmul (weight fold done on device).
  - Host: slot table st[slot,32] = x[src](14) | as[src](8) | ad[dst](8)
    | rank_rel | first   (fp16), rank relative to its subchunk's window
    base (baked into the program).
  - L1: per batch of 4 superchunks: DVE softmax weights ew (max
    subtraction dropped - logits are bounded and per-segment constants
    cancel in segment softmax), Pool builds xw = [ew (x) x_src | first |
    ew], W-wide one-hot scatter matmuls accumulate into PSUM at baked
    partition offsets (zeroing matmul first), the one-hot is written to
    DRAM for L2 reuse, and the node phase CONSUMES PSUM DIRECTLY: 1/s
    normalize, transpose, out1 = ELU(T@W1blk+b1) via the exp/min/max
    trick, contract to [as2|ad2|h2] = 6 values per rank row.
  - Host: layer-2 slot table st6 = as2[src]|ad2[dst]|h2[src]|rank_rel.
  - L2: same chunk machinery, 6-wide, reusing the stored one-hot:
    segment softmax + scatter + 1/s + b2; host unpermutes rank rows.
"""
import os
import shutil
import sys

sys.path.insert(0, "/opt/trn_rl_repo")

import numpy as np

import concourse.bacc as bacc
import concourse.mybir as mybir
import concourse.tile as tile
from concourse.bass_utils import run_bass_kernel_spmd

P = 128
IC = 14          # input channels
H = 8            # heads (layer 1)
F = 128          # per-head features (layer 1)
D1 = H * F       # 1024
O2 = 4           # layer-2 out dim
NEG = 0.2

SUBS = 5         # subchunks per superchunk
SCS = SUBS * P   # 640 slots per superchunk
RK = 128         # max dst-groups (ranks) per superchunk
SPB = 4          # superchunks per batch
G = SPB * SUBS   # 20 subchunks per batch
BS = SPB * P     # 512 rank rows per batch
TW = 121         # 112 T cols + 1 first col + 8 s cols
W = 40           # one-hot window width (per-subchunk rank span)
Q = 22           # fixed rank-window stride: subchunk j covers [j*Q, j*Q+W)

F32 = mybir.dt.float32
F32R = mybir.dt.float32r
FP16 = mybir.dt.float16

N_CORES = 8

_trace = bool(os.environ.get("GAT_TRACE"))
_trace_dir = os.environ.get("GAT_TRACE_DIR", "/tmp/gat_trace")


# ----------------------------------------------------------------- host pack
def pack_graph(edge_index, n_nodes):
    e0 = np.asarray(edge_index[0], dtype=np.int64)
    e1 = np.asarray(edge_index[1], dtype=np.int64)
    loops = np.arange(n_nodes, dtype=np.int64)
    src = np.concatenate([e0, loops])
    dst = np.concatenate([e1, loops])

    order = np.argsort(dst, kind="stable")
    src = src[order]
    dst = dst[order]
    grp_starts = np.flatnonzero(np.r_[True, dst[1:] != dst[:-1]])
    grp_sizes = np.diff(np.r_[grp_starts, dst.size]).astype(np.int64)
    n_groups = grp_starts.size
    assert n_groups == n_nodes
    assert grp_sizes.max() <= P

    # superchunk packing: whole groups, <=SCS slots, <=RK groups, and
    # subchunk j only holds groups whose rank lies in [j*Q, j*Q+W)
    # (universal fixed windows -> one SPMD program for all cores)
    chunk_of_group = np.empty(n_groups, np.int64)
    slot0_of_group = np.empty(n_groups, np.int64)
    rank_of_group = np.empty(n_groups, np.int64)
    ci = 0
    used = 0
    rk = 0
    for g in range(n_groups):
        sz = grp_sizes[g]
        while True:
            if used + sz > SCS or rk >= RK:
                ci += 1
                used = 0
                rk = 0
                continue
            j_hi = rk // Q
            j_lo = max(0, -(-(rk - W + 1) // Q))
            jc = used // P
            if jc > j_hi or (used + sz - 1) // P > j_hi:
                ci += 1
                used = 0
                rk = 0
                continue
            if jc < j_lo:
                used = j_lo * P
                continue
            break
        chunk_of_group[g] = ci
        slot0_of_group[g] = used
        rank_of_group[g] = rk
        used += sz
        rk += 1
    n_sc = ci + 1

    gid = np.repeat(np.arange(n_groups), grp_sizes)
    within = np.arange(dst.size) - np.repeat(grp_starts, grp_sizes)
    e_chunk = chunk_of_group[gid]
    e_slot = slot0_of_group[gid] + within
    e_rank = rank_of_group[gid]
    e_first = (within == 0)

    per_core_sc = -(-n_sc // N_CORES)
    scmax = -(-per_core_sc // SPB) * SPB

    # relative rank per edge slot (fixed windows w0_j = j*Q)
    e_rel = e_rank - Q * (e_slot // P)
    assert e_rel.min() >= 0 and e_rel.max() < W

    first_group_of_chunk = np.zeros(n_sc + 1, np.int64)
    fg = np.flatnonzero(np.r_[True, chunk_of_group[1:] != chunk_of_group[:-1]])
    first_group_of_chunk[:n_sc] = fg
    first_group_of_chunk[n_sc] = n_groups

    per_core = []
    for k in range(N_CORES):
        lo = min(k * per_core_sc, n_sc)
        hi = min(lo + per_core_sc, n_sc)
        gfirst = first_group_of_chunk[lo] if lo < n_sc else n_groups
        glast = first_group_of_chunk[hi] if hi < n_sc else n_groups
        nloc = int(glast - gfirst)
        dstbase = int(dst[grp_starts[gfirst]]) if gfirst < n_groups else n_nodes

        nslots = scmax * SCS
        srcg = np.zeros(nslots, np.int64)
        dstg = np.zeros(nslots, np.int64)
        rel = np.full(nslots, -1.0, np.float32)
        first = np.zeros(nslots, np.float32)

        sel = (e_chunk >= lo) & (e_chunk < hi)
        ss = (e_chunk[sel] - lo) * SCS + e_slot[sel]
        srcg[ss] = src[sel]
        dstg[ss] = dst[sel]
        rel[ss] = e_rel[sel]
        first[ss] = e_first[sel].astype(np.float32)

        gsel = (chunk_of_group >= lo) & (chunk_of_group < hi)
        # rank-row index (node-sorted) of each of this core's nodes
        nodecr = ((chunk_of_group[gsel] - lo) * RK
                  + rank_of_group[gsel]).astype(np.int64)

        per_core.append(dict(
            nloc=nloc, dstbase=dstbase, srcg=srcg, dstg=dstg, nodecr=nodecr,
            rel=rel, first=first,
        ))

    return dict(per_core=per_core, scmax=scmax, n_sc=n_sc)


def host_weights(W1, att_src1, att_dst1, b1):
    """Pure re-layouts of weights (no data-sized arithmetic)."""
    W1 = np.asarray(W1, np.float32)
    W1T = np.ascontiguousarray(W1.T)                       # [D1, IC]
    Ablk = np.zeros((D1, 2 * H), np.float32)
    a_s = np.asarray(att_src1, np.float32)
    a_d = np.asarray(att_dst1, np.float32)
    for h in range(H):
        Ablk[h * F:(h + 1) * F, h] = a_s[h]
        Ablk[h * F:(h + 1) * F, H + h] = a_d[h]
    W1b = np.zeros((P, D1), np.float16)
    for h in range(H):
        W1b[9 + h * IC:9 + (h + 1) * IC, h * F:(h + 1) * F] = (
            W1[:, h * F:(h + 1) * F].astype(np.float16))
    W1b[8] = (np.asarray(b1, np.float32) + 1.0).astype(np.float16)
    # E9 [9, 121]: expands [1/s_h (8 rows) | 1] to per-xw-column scales
    E9 = np.zeros((9, TW), np.float16)
    for h in range(H):
        E9[h, 9 + h * IC:9 + (h + 1) * IC] = 1.0
    E9[8, 0:9] = 1.0
    return W1T, Ablk, W1b, E9


# ------------------------------------------------------------- launch 0 bass
def build_l0(npc):
    """Per-node asadT[16, npc] = ([W1@a_src | W1@a_dst].T) @ xT."""
    nt = npc // 512
    nc = bacc.Bacc("TRN2", target_bir_lowering=False)

    xt_in = nc.dram_tensor("xt", [IC, npc], FP16, kind="ExternalInput")
    w1t_in = nc.dram_tensor("w1t", [D1, IC], F32, kind="ExternalInput")
    ablk_in = nc.dram_tensor("ablk", [D1, 2 * H], F32, kind="ExternalInput")
    asadt_out = nc.dram_tensor("asadt", [2 * H, npc], FP16,
                               kind="ExternalOutput")

    with tile.TileContext(nc) as tc:
        with (
            tc.tile_pool(name="const", bufs=1) as cpool,
            tc.tile_pool(name="work", bufs=3) as wpool,
            tc.tile_pool(name="ps", bufs=4, space="PSUM") as ps,
        ):
            # fold AsAd[14, 16] = sum_b W1T_b.T @ Ablk_b
            w1t_t = wpool.tile([P, H, IC], F32, tag="w1tt")
            nc.sync.dma_start(
                out=w1t_t[:], in_=w1t_in.rearrange("(b p) k -> p b k", p=P))
            ablk_t = wpool.tile([P, H, 2 * H], F32, tag="ablkt")
            nc.sync.dma_start(
                out=ablk_t[:], in_=ablk_in.rearrange("(b p) k -> p b k", p=P))
            asad_ps = ps.tile([IC, 2 * H], F32, tag="fold", bufs=1)
            for b in range(H):
                nc.tensor.matmul(out=asad_ps[:], lhsT=w1t_t[:, b, :],
                                 rhs=ablk_t[:, b, :], start=(b == 0),
                                 stop=(b == H - 1))
            asad_h = cpool.tile([IC, 2 * H], FP16, tag="asadh")
            with nc.allow_low_precision(reason="fp16 logit weights"):
                nc.vector.tensor_copy(out=asad_h[:], in_=asad_ps[:])

            xt = cpool.tile([IC, npc], FP16, tag="xt")
            nc.sync.dma_start(out=xt[:], in_=xt_in[:, :])
            asb = cpool.tile([2 * H, npc], FP16, tag="asb")
            for t in range(nt):
                a_ps = ps.tile([2 * H, 512], F32, tag="aps", bufs=4)
                nc.tensor.matmul(out=a_ps[:], lhsT=asad_h[:],
                                 rhs=xt[:, t * 512:(t + 1) * 512],
                                 start=True, stop=True)
                with nc.allow_low_precision(reason="fp16 logits"):
                    if t % 2 == 0:
                        nc.scalar.copy(out=asb[:, t * 512:(t + 1) * 512],
                                       in_=a_ps[:])
                    else:
                        nc.vector.tensor_copy(
                            out=asb[:, t * 512:(t + 1) * 512], in_=a_ps[:])
            nc.sync.dma_start(out=asadt_out[:, :], in_=asb[:])

    nc.compile()
    return nc


# ------------------------------------------------------------- launch 1 bass
def build_l1(scmax):
    nb = scmax // SPB
    nc = bacc.Bacc("TRN2", target_bir_lowering=False)

    st_in = nc.dram_tensor("st", [P, scmax * SUBS, 32], FP16,
                           kind="ExternalInput")
    w1b_in = nc.dram_tensor("w1b", [P, D1], FP16, kind="ExternalInput")
    w2_in = nc.dram_tensor("w2", [D1, O2], F32, kind="ExternalInput")
    att2t_in = nc.dram_tensor("att2t", [O2, 2], F32, kind="ExternalInput")
    iota_in = nc.dram_tensor("iota", [P, W], FP16, kind="ExternalInput")
    identf_in = nc.dram_tensor("identf", [P, P], F32, kind="ExternalInput")
    e9_in = nc.dram_tensor("e9", [9, TW], FP16, kind="ExternalInput")
    t2rawt = nc.dram_tensor("t2rawt", [6, scmax * RK], F32,
                            kind="ExternalOutput")

    with tile.TileContext(nc) as tc:
        with (
            tc.tile_pool(name="const", bufs=1) as cpool,
            tc.tile_pool(name="work", bufs=3) as wpool,
            tc.tile_pool(name="chunk", bufs=4) as kpool,
            tc.tile_pool(name="ps", bufs=8, space="PSUM") as ps,
        ):
            identf = cpool.tile([P, P], F32, tag="identf")
            nc.sync.dma_start(out=identf[:], in_=identf_in[:, :])
            e9 = cpool.tile([9, TW], FP16, tag="e9")
            nc.sync.dma_start(out=e9[:], in_=e9_in[:, :])
            iota = cpool.tile([P, W], FP16, tag="iota")
            nc.sync.dma_start(out=iota[:], in_=iota_in[:, :])
            neg1 = cpool.tile([P, 1], F32, tag="neg1")
            nc.vector.memset(neg1[:], -1.0)
            ones = cpool.tile([P, 1], F32, tag="ones")
            nc.vector.memset(ones[:], 1.0)
            w1b = cpool.tile([P, D1], FP16, tag="w1b")
            nc.sync.dma_start(out=w1b[:], in_=w1b_in[:, :])

            # rhs6 [128, H, 6] = [va_b | vd_b | W2_b] fp16; -r6s [6,1]
            att2t_t = cpool.tile([O2, 2], F32, tag="att2t")
            nc.sync.dma_start(out=att2t_t[:], in_=att2t_in[:, :])
            rhs6f = wpool.tile([P, H, 6], F32, tag="rhs6f")
            for b in range(H):
                w2b = wpool.tile([P, O2], F32, tag="w2b")
                nc.sync.dma_start(out=w2b[:], in_=w2_in[b * P:(b + 1) * P, :])
                nc.vector.tensor_copy(out=rhs6f[:, b, 2:2 + O2], in_=w2b[:])
                w2bt_ps = ps.tile([O2, P], F32, tag="scr", bufs=1)
                nc.tensor.transpose(out=w2bt_ps[:], in_=w2b[:],
                                    identity=identf[:])
                w2bt = wpool.tile([O2, P], F32, tag="w2bt")
                nc.scalar.copy(out=w2bt[:], in_=w2bt_ps[:])
                vavd_ps = ps.tile([P, 2], F32, tag="scr", bufs=1)
                nc.tensor.matmul(out=vavd_ps[:], lhsT=w2bt[:], rhs=att2t_t[:],
                                 start=True, stop=True)
                nc.vector.tensor_copy(out=rhs6f[:, b, 0:2], in_=vavd_ps[:])
            rhs6 = cpool.tile([P, H, 6], FP16, tag="rhs6")
            with nc.allow_low_precision(reason="fp16 contraction weights"):
                nc.vector.tensor_copy(out=rhs6[:], in_=rhs6f[:])
            r6_ps = ps.tile([6, 1], F32, tag="acc", bufs=2)
            for b in range(H):
                nc.tensor.matmul(out=r6_ps[:], lhsT=rhs6f[:, b, :],
                                 rhs=ones[:], start=(b == 0), stop=(b == H - 1))
            negr6s = cpool.tile([6, 1], F32, tag="negr6s")
            nc.vector.tensor_scalar(out=negr6s[:], in0=r6_ps[:], scalar1=-1.0,
                                    scalar2=None, op0=mybir.AluOpType.mult)

            # whole slot table resident
            stall = cpool.tile([P, scmax * SUBS, 32], FP16, tag="stall")
            nc.sync.dma_start(out=stall[:], in_=st_in[:, :, :])

            # persistent full-width one-hot tiles; pad columns zeroed once,
            # only the fixed per-subchunk windows [Q*j, Q*j+W) are rewritten
            # each batch (single strided-AP is_equal per batch)
            oh_bufs = []
            for i in range(3):
                ohf = cpool.tile([P, G, P], FP16, tag=f"ohf{i}")
                nc.gpsimd.memset(ohf[:], 0.0)
                oh_bufs.append(ohf)
            rcp9_bufs = []
            for i in range(3):
                r9 = cpool.tile([9, SPB, P], FP16, tag=f"rcp9{i}")
                nc.vector.memset(r9[:], 1.0)
                rcp9_bufs.append(r9)

            def _winap(ohf):
                ap = ohf[:].copy()
                ap.ap = type(ap.ap)(
                    [[G * P, P], [SUBS * P, SPB], [P + Q, SUBS], [1, W]])
                return ap

            def batch(b):
                stb = stall[:, b * G:(b + 1) * G, :]
                z = kpool.tile([P, G, H], FP16, tag="z")
                nc.gpsimd.tensor_tensor(out=z[:], in0=stb[:, :, IC:IC + H],
                                        in1=stb[:, :, IC + H:IC + 2 * H],
                                        op=mybir.AluOpType.add)
                lg = kpool.tile([P, G, H], FP16, tag="lg")
                nc.vector.scalar_tensor_tensor(
                    out=lg[:], in0=z[:], scalar=NEG, in1=z[:],
                    op0=mybir.AluOpType.mult, op1=mybir.AluOpType.max)
                ew = kpool.tile([P, G, H], FP16, tag="ew")
                nc.scalar.activation(ew[:], lg[:],
                                     mybir.ActivationFunctionType.Exp)
                ohf = oh_bufs[b % 3]
                nc.vector.tensor_tensor(
                    out=_winap(ohf),
                    in0=iota[:].rearrange("p (a c r) -> p a c r", a=1, c=1)
                        .to_broadcast([P, SPB, SUBS, W]),
                    in1=stb[:, :, 30:31].rearrange(
                        "p (s j) k -> p s j k", j=SUBS)
                        .to_broadcast([P, SPB, SUBS, W]),
                    op=mybir.AluOpType.is_equal)
                xw = kpool.tile([P, G, TW], FP16, tag="xw")
                for s_ in range(SPB):
                    c0 = s_ * SUBS
                    nc.gpsimd.tensor_tensor(
                        out=xw[:, c0:c0 + SUBS, 9:TW].rearrange(
                            "p g (h k) -> p g h k", h=H),
                        in0=stb[:, c0:c0 + SUBS, 0:IC].rearrange(
                            "p g (a k) -> p g a k", a=1)
                            .to_broadcast([P, SUBS, H, IC]),
                        in1=ew[:, c0:c0 + SUBS, :].rearrange(
                            "p g (h a) -> p g h a", a=1)
                            .to_broadcast([P, SUBS, H, IC]),
                        op=mybir.AluOpType.mult)
                nc.gpsimd.tensor_copy(out=xw[:, :, 0:H], in_=ew[:])
                nc.gpsimd.tensor_copy(out=xw[:, :, H:H + 1],
                                      in_=stb[:, :, 31:32])

                # flipped scatter: t2T[xw-col, rank] accumulates per sc
                t2_ps = ps.tile([TW, SPB, P], F32, tag="t2ps", bufs=2)
                for s in range(SPB):
                    for j in range(SUBS):
                        c = s * SUBS + j
                        nc.tensor.matmul(out=t2_ps[:, s, :],
                                         lhsT=xw[:, c, :],
                                         rhs=ohf[:, c, :],
                                         start=(j == 0),
                                         stop=(j == SUBS - 1))
                # node phase, transposed: rcp9 rows 0:8 = 1/s, row 8 = 1
                r32 = wpool.tile([8, SPB, P], F32, tag="r32")
                nc.vector.reciprocal_approx_fast(out=r32[:],
                                                 in_=t2_ps[0:8, :, :])
                rcp9 = rcp9_bufs[b % 3]
                with nc.allow_low_precision(reason="fp16 1/s"):
                    nc.vector.tensor_copy(out=rcp9[0:8, :, :], in_=r32[:])
                rc_ps = ps.tile([TW, SPB, P], F32, tag="scr", bufs=1)
                nc.tensor.matmul(out=rc_ps[:, :, :], lhsT=e9[:],
                                 rhs=rcp9[:, :, :], start=True, stop=True)
                rcs = wpool.tile([TW, SPB, P], FP16, tag="rcs")
                with nc.allow_low_precision(reason="fp16 scales"):
                    nc.scalar.copy(out=rcs[:], in_=rc_ps[:])
                tnt = kpool.tile([TW, SPB, P], FP16, tag="tnt")
                nc.vector.tensor_tensor(out=tnt[:], in0=t2_ps[:],
                                        in1=rcs[:],
                                        op=mybir.AluOpType.mult)

                h2_ps = ps.tile([6, BS], F32, tag="acc", bufs=2)
                for h in range(H):
                    o1_ps = ps.tile([P, BS], F32, tag="o1ps", bufs=3)
                    nc.tensor.matmul(out=o1_ps[:],
                                     lhsT=w1b[0:TW, h * F:(h + 1) * F],
                                     rhs=tnt[:, :, :], start=True, stop=True)
                    e = wpool.tile([P, BS], FP16, tag="e")
                    nc.scalar.activation(e[:], o1_ps[:],
                                         mybir.ActivationFunctionType.Exp,
                                         bias=neg1[:, 0:1])
                    e1 = wpool.tile([P, BS], FP16, tag="e1")
                    nc.vector.scalar_tensor_tensor(
                        out=e1[:], in0=e[:], scalar=1.0, in1=o1_ps[:],
                        op0=mybir.AluOpType.min, op1=mybir.AluOpType.max)
                    nc.tensor.matmul(out=h2_ps[:], lhsT=rhs6[:, h, :],
                                     rhs=e1[:], start=(h == 0),
                                     stop=(h == H - 1))
                h2sb = wpool.tile([6, BS], F32, tag="h2sb")
                nc.scalar.activation(h2sb[:], h2_ps[:],
                                     mybir.ActivationFunctionType.Identity,
                                     bias=negr6s[:, 0:1])
                nc.sync.dma_start(out=t2rawt[:, b * BS:(b + 1) * BS],
                                  in_=h2sb[:])

            for b in range(nb):
                batch(b)

    nc.compile()
    return nc


# ------------------------------------------------------------- launch 2 bass
def build_l2(scmax):
    nb = scmax // SPB
    nc = bacc.Bacc("TRN2", target_bir_lowering=False)

    st_in = nc.dram_tensor("st6", [P, scmax * SUBS, 8], FP16,
                           kind="ExternalInput")
    iota_in = nc.dram_tensor("iota", [P, W], FP16, kind="ExternalInput")
    b2t_in = nc.dram_tensor("b2t", [O2, 1], F32, kind="ExternalInput")
    identf_in = nc.dram_tensor("identf", [P, P], F32, kind="ExternalInput")
    out2 = nc.dram_tensor("out2", [P, scmax // SPB, SPB, O2], F32,
                          kind="ExternalOutput")

    with tile.TileContext(nc) as tc:
        with (
            tc.tile_pool(name="const", bufs=1) as cpool,
            tc.tile_pool(name="chunk", bufs=4) as kpool,
            tc.tile_pool(name="ps", bufs=8, space="PSUM") as ps,
        ):
            identf = cpool.tile([P, P], F32, tag="identf")
            nc.sync.dma_start(out=identf[:], in_=identf_in[:, :])
            b2t_t = cpool.tile([O2, 1], F32, tag="b2t")
            nc.sync.dma_start(out=b2t_t[:], in_=b2t_in[:, :])
            b2_ps = ps.tile([P, O2], F32, tag="setup", bufs=1)
            nc.tensor.transpose(out=b2_ps[:],
                                in_=b2t_t[:].to_broadcast([O2, P]),
                                identity=identf[0:O2, 0:O2])
            b2bc = cpool.tile([P, O2], F32, tag="b2bc")
            nc.vector.tensor_copy(out=b2bc[:], in_=b2_ps[:])
            iota = cpool.tile([P, W], FP16, tag="iota")
            nc.sync.dma_start(out=iota[:], in_=iota_in[:, :])

            stall = cpool.tile([P, scmax * SUBS, 8], FP16, tag="stall")
            nc.sync.dma_start(out=stall[:], in_=st_in[:, :, :])
            outsb = cpool.tile([P, nb, SPB, O2], F32, tag="outsb")

            oh_bufs = []
            for i in range(3):
                ohf = cpool.tile([P, G, P], FP16, tag=f"ohf{i}")
                nc.gpsimd.memset(ohf[:], 0.0)
                oh_bufs.append(ohf)
            rcp9_bufs = []
            for i in range(3):
                r9 = cpool.tile([9, SPB, P], FP16, tag=f"rcp9{i}")
                nc.vector.memset(r9[:], 1.0)
                rcp9_bufs.append(r9)

            def _winap(ohf):
                ap = ohf[:].copy()
                ap.ap = type(ap.ap)(
                    [[G * P, P], [SUBS * P, SPB], [P + Q, SUBS], [1, W]])
                return ap

            def batch(b):
                stb = stall[:, b * G:(b + 1) * G, :]
                z = kpool.tile([P, G, 1], FP16, tag="z")
                nc.gpsimd.tensor_tensor(out=z[:], in0=stb[:, :, 0:1],
                                        in1=stb[:, :, 1:2],
                                        op=mybir.AluOpType.add)
                lg = kpool.tile([P, G, 1], FP16, tag="lg")
                nc.vector.scalar_tensor_tensor(
                    out=lg[:], in0=z[:], scalar=NEG, in1=z[:],
                    op0=mybir.AluOpType.mult, op1=mybir.AluOpType.max)
                ew = kpool.tile([P, G, 1], FP16, tag="ew")
                nc.scalar.activation(ew[:], lg[:],
                                     mybir.ActivationFunctionType.Exp)
                r5 = kpool.tile([P, G, O2 + 1], FP16, tag="r5")
                nc.gpsimd.tensor_tensor(
                    out=r5[:, :, 0:O2], in0=stb[:, :, 2:6],
                    in1=ew[:].to_broadcast([P, G, O2]),
                    op=mybir.AluOpType.mult)
                nc.vector.tensor_copy(out=r5[:, :, O2:O2 + 1], in_=ew[:])

                ohf = oh_bufs[b % 3]
                nc.vector.tensor_tensor(
                    out=_winap(ohf),
                    in0=iota[:].rearrange("p (a c r) -> p a c r", a=1, c=1)
                        .to_broadcast([P, SPB, SUBS, W]),
                    in1=stb[:, :, 6:7].rearrange(
                        "p (s j) k -> p s j k", j=SUBS)
                        .to_broadcast([P, SPB, SUBS, W]),
                    op=mybir.AluOpType.is_equal)
                o5_ps = ps.tile([P, SPB, O2 + 1], F32, tag="o5ps", bufs=3)
                for s in range(SPB):
                    for j in range(SUBS):
                        c = s * SUBS + j
                        nc.tensor.matmul(out=o5_ps[:, s, :],
                                         lhsT=ohf[:, c, :],
                                         rhs=r5[:, c, :],
                                         start=(j == 0),
                                         stop=(j == SUBS - 1))
                rcp = kpool.tile([P, SPB, 1], F32, tag="rcp")
                nc.vector.reciprocal(out=rcp[:], in_=o5_ps[:, :, O2:O2 + 1])
                of = kpool.tile([P, SPB, O2], F32, tag="of")
                nc.vector.tensor_tensor(out=of[:], in0=o5_ps[:, :, 0:O2],
                                        in1=rcp[:].to_broadcast(
                                            [P, SPB, O2]),
                                        op=mybir.AluOpType.mult)
                nc.gpsimd.tensor_tensor(
                    out=outsb[:, b, :, :], in0=of[:],
                    in1=b2bc[:].rearrange("p (a k) -> p a k", a=1)
                        .to_broadcast([P, SPB, O2]),
                    op=mybir.AluOpType.add)

            for b in range(nb):
                batch(b)
            nc.sync.dma_start(out=out2[:, :, :, :], in_=outsb[:])

    nc.compile()
    return nc


# ------------------------------------------------------------------- kernel
_cache = {}
_pack_cache = {}
LAST_EXEC_NS = []  # [l0, l1, l2] when GAT_TRACE=1


def kernel(x, edge_index, W1, att_src1, att_dst1, b1, W2, att_src2,
           att_dst2, b2):
    x = np.asarray(x, np.float32)
    n_nodes = x.shape[0]
    ei = np.asarray(edge_index)

    pkey = (hash(ei.tobytes()), n_nodes)
    if pkey not in _pack_cache:
        _pack_cache[pkey] = pack_graph(ei, n_nodes)
    pk = _pack_cache[pkey]
    scmax = pk["scmax"]

    npc0 = -(-max(-(-n_nodes // N_CORES), 1) // 512) * 512

    key = (n_nodes, scmax, npc0)
    if key not in _cache:
        _cache[key] = (build_l0(npc0), build_l1(scmax), build_l2(scmax))
    nc0, nc1, nc2 = _cache[key]

    W1T, Ablk, W1b, E9 = host_weights(W1, att_src1, att_dst1, b1)
    att2T = np.stack([np.asarray(att_src2, np.float32)[0],
                      np.asarray(att_dst2, np.float32)[0]], axis=1)  # [4, 2]
    iota = np.tile(np.arange(W, dtype=np.float16)[None, :], (P, 1))
    identf = np.eye(P, dtype=np.float32)

    x16 = x.astype(np.float16)

    if _trace:
        for sfx in ("_l0", "_l1", "_l2"):
            shutil.rmtree(_trace_dir + sfx, ignore_errors=True)
    LAST_EXEC_NS.clear()

    # ---- L0: per-node attention logits
    npern = -(-n_nodes // N_CORES)
    in_maps0 = []
    for k in range(N_CORES):
        lo = k * npern
        hi = min(lo + npern, n_nodes)
        xt = np.zeros((IC, npc0), np.float16)
        xt[:, 0:hi - lo] = x16[lo:hi].T
        in_maps0.append({"xt": xt, "w1t": W1T, "ablk": Ablk})
    res0 = run_bass_kernel_spmd(
        nc0, in_maps0, list(range(N_CORES)), trace=_trace,
        tmpdir=(_trace_dir + "_l0") if _trace else None)
    if _trace:
        print("L0 exec_time_ns:", res0.exec_time_ns)
        LAST_EXEC_NS.append(res0.exec_time_ns)
    asad = np.empty((n_nodes, 2 * H), np.float16)
    for k in range(N_CORES):
        lo = k * npern
        hi = min(lo + npern, n_nodes)
        asad[lo:hi] = res0.results[k]["asadt"][:, 0:hi - lo].T

    # ---- L1: edge pass 1 + node pass
    in_maps1 = []
    for k in range(N_CORES):
        pc = pk["per_core"][k]
        sflat = pc["srcg"]
        dflat = pc["dstg"]
        st = np.empty((scmax * SCS, 32), np.float16)
        st[:, 0:IC] = x16[sflat]
        st[:, IC:IC + H] = asad[sflat, 0:H]
        st[:, IC + H:IC + 2 * H] = asad[dflat, H:2 * H]
        st[:, 30] = pc["rel"]
        st[:, 31] = pc["first"]
        st = np.ascontiguousarray(
            st.reshape(scmax * SUBS, P, 32).transpose(1, 0, 2))
        in_maps1.append({
            "st": st, "w1b": W1b, "w2": np.asarray(W2, np.float32),
            "att2t": att2T, "iota": iota, "identf": identf, "e9": E9,
        })
    res1 = run_bass_kernel_spmd(
        nc1, in_maps1, list(range(N_CORES)), trace=_trace,
        tmpdir=(_trace_dir + "_l1") if _trace else None)
    if _trace:
        print("L1 exec_time_ns:", res1.exec_time_ns)
        LAST_EXEC_NS.append(res1.exec_time_ns)

    # host assembly: rank rows -> node rows
    h2full = np.empty((n_nodes, 6), np.float32)
    for k in range(N_CORES):
        pc = pk["per_core"][k]
        h2full[pc["dstbase"]:pc["dstbase"] + pc["nloc"]] = (
            res1.results[k]["t2rawt"][:, pc["nodecr"]].T)
    h2f16 = h2full.astype(np.float16)

    # ---- L2: edge pass 2
    in_maps2 = []
    for k in range(N_CORES):
        pc = pk["per_core"][k]
        sflat = pc["srcg"]
        dflat = pc["dstg"]
        st6 = np.zeros((scmax * SCS, 8), np.float16)
        st6[:, 0] = h2f16[sflat, 0]              # as2[src]
        st6[:, 1] = h2f16[dflat, 1]              # ad2[dst]
        st6[:, 2:6] = h2f16[sflat, 2:6]          # h2[src]
        st6[:, 6] = pc["rel"]
        st6 = np.ascontiguousarray(
            st6.reshape(scmax * SUBS, P, 8).transpose(1, 0, 2))
        in_maps2.append({
            "st6": st6, "iota": iota, "identf": identf,
            "b2t": np.asarray(b2, np.float32).reshape(O2, 1),
        })
    res2 = run_bass_kernel_spmd(
        nc2, in_maps2, list(range(N_CORES)), trace=_trace,
        tmpdir=(_trace_dir + "_l2") if _trace else None)
    if _trace:
        print("L2 exec_time_ns:", res2.exec_time_ns)
        LAST_EXEC_NS.append(res2.exec_time_ns)

    out = np.zeros((n_nodes, O2), np.float32)
    for k in range(N_CORES):
        pc = pk["per_core"][k]
        o = res2.results[k]["out2"].transpose(1, 2, 0, 3).reshape(-1, O2)
        out[pc["dstbase"]:pc["dstbase"] + pc["nloc"]] = o[pc["nodecr"]]
    return out


# revision 26
# speedup vs baseline: 1.0687x; 1.0687x over previous
"""Trainium2 Bass kernel for a 2-layer GAT (nn_GAT_82901458747986).

Strategy (8-core SPMD, 3 launches, zero per-edge device gathers, no
DRAM roundtrip for the aggregation table):
  - Host: add self-loops, sort edges by dst, pack whole dst-groups into
    superchunks of <=640 edge slots (5 subchunks of 128) and <=128
    groups, keeping each 128-slot subchunk's rank span <= W=40; assign
    contiguous superchunk ranges to cores.  Per-edge tables are
    host-built by pure permutation (the "halo exchange").
  - L0: per-node attention logits asad[n,16] = x @ [W1@a_src | W1@a_dst]
    via one streamed matmul (weight fold done on device).
  - Host: slot table st[slot,32] = x[src](14) | as[src](8) | ad[dst](8)
    | rank_rel | first   (fp16), rank relative to its subchunk's window
    base (baked into the program).
  - L1: per batch of 4 superchunks: DVE softmax weights ew (max
    subtraction dropped - logits are bounded and per-segment constants
    cancel in segment softmax), Pool builds xw = [ew (x) x_src | first |
    ew], W-wide one-hot scatter matmuls accumulate into PSUM at baked
    partition offsets (zeroing matmul first), the one-hot is written to
    DRAM for L2 reuse, and the node phase CONSUMES PSUM DIRECTLY: 1/s
    normalize, transpose, out1 = ELU(T@W1blk+b1) via the exp/min/max
    trick, contract to [as2|ad2|h2] = 6 values per rank row.
  - Host: layer-2 slot table st6 = as2[src]|ad2[dst]|h2[src]|rank_rel.
  - L2: same chunk machinery, 6-wide, reusing the stored one-hot:
    segment softmax + scatter + 1/s + b2; host unpermutes rank rows.
"""
import os
import shutil
import sys

sys.path.insert(0, "/opt/trn_rl_repo")

import numpy as np

import concourse.bacc as bacc
import concourse.mybir as mybir
import concourse.tile as tile
from concourse.bass_utils import run_bass_kernel_spmd

P = 128
IC = 14          # input channels
H = 8            # heads (layer 1)
F = 128          # per-head features (layer 1)
D1 = H * F       # 1024
O2 = 4           # layer-2 out dim
NEG = 0.2

SUBS = 5         # subchunks per superchunk
SCS = SUBS * P   # 640 slots per superchunk
RK = 128         # max dst-groups (ranks) per superchunk
SPB = 4          # superchunks per batch
G = SPB * SUBS   # 20 subchunks per batch
BS = SPB * P     # 512 rank rows per batch
TW = 121         # 112 T cols + 1 first col + 8 s cols
W = 40           # one-hot window width (per-subchunk rank span)
Q = 22           # fixed rank-window stride: subchunk j covers [j*Q, j*Q+W)

F32 = mybir.dt.float32
F32R = mybir.dt.float32r
FP16 = mybir.dt.float16

N_CORES = 8

_trace = bool(os.environ.get("GAT_TRACE"))
_trace_dir = os.environ.get("GAT_TRACE_DIR", "/tmp/gat_trace")


# ----------------------------------------------------------------- host pack
def pack_graph(edge_index, n_nodes):
    e0 = np.asarray(edge_index[0], dtype=np.int64)
    e1 = np.asarray(edge_index[1], dtype=np.int64)
    loops = np.arange(n_nodes, dtype=np.int64)
    src = np.concatenate([e0, loops])
    dst = np.concatenate([e1, loops])

    order = np.argsort(dst, kind="stable")
    src = src[order]
    dst = dst[order]
    grp_starts = np.flatnonzero(np.r_[True, dst[1:] != dst[:-1]])
    grp_sizes = np.diff(np.r_[grp_starts, dst.size]).astype(np.int64)
    n_groups = grp_starts.size
    assert n_groups == n_nodes
    assert grp_sizes.max() <= P

    # superchunk packing: whole groups, <=SCS slots, <=RK groups, and
    # subchunk j only holds groups whose rank lies in [j*Q, j*Q+W)
    # (universal fixed windows -> one SPMD program for all cores)
    chunk_of_group = np.empty(n_groups, np.int64)
    slot0_of_group = np.empty(n_groups, np.int64)
    rank_of_group = np.empty(n_groups, np.int64)
    ci = 0
    used = 0
    rk = 0
    for g in range(n_groups):
        sz = grp_sizes[g]
        while True:
            if used + sz > SCS or rk >= RK:
                ci += 1
                used = 0
                rk = 0
                continue
            j_hi = rk // Q
            j_lo = max(0, -(-(rk - W + 1) // Q))
            jc = used // P
            if jc > j_hi or (used + sz - 1) // P > j_hi:
                ci += 1
                used = 0
                rk = 0
                continue
            if jc < j_lo:
                used = j_lo * P
                continue
            break
        chunk_of_group[g] = ci
        slot0_of_group[g] = used
        rank_of_group[g] = rk
        used += sz
        rk += 1
    n_sc = ci + 1

    gid = np.repeat(np.arange(n_groups), grp_sizes)
    within = np.arange(dst.size) - np.repeat(grp_starts, grp_sizes)
    e_chunk = chunk_of_group[gid]
    e_slot = slot0_of_group[gid] + within
    e_rank = rank_of_group[gid]
    e_first = (within == 0)

    per_core_sc = -(-n_sc // N_CORES)
    scmax = -(-per_core_sc // SPB) * SPB

    # relative rank per edge slot (fixed windows w0_j = j*Q)
    e_rel = e_rank - Q * (e_slot // P)
    assert e_rel.min() >= 0 and e_rel.max() < W

    first_group_of_chunk = np.zeros(n_sc + 1, np.int64)
    fg = np.flatnonzero(np.r_[True, chunk_of_group[1:] != chunk_of_group[:-1]])
    first_group_of_chunk[:n_sc] = fg
    first_group_of_chunk[n_sc] = n_groups

    per_core = []
    for k in range(N_CORES):
        lo = min(k * per_core_sc, n_sc)
        hi = min(lo + per_core_sc, n_sc)
        gfirst = first_group_of_chunk[lo] if lo < n_sc else n_groups
        glast = first_group_of_chunk[hi] if hi < n_sc else n_groups
        nloc = int(glast - gfirst)
        dstbase = int(dst[grp_starts[gfirst]]) if gfirst < n_groups else n_nodes

        nslots = scmax * SCS
        srcg = np.zeros(nslots, np.int64)
        dstg = np.zeros(nslots, np.int64)
        rel = np.full(nslots, -1.0, np.float32)
        first = np.zeros(nslots, np.float32)

        sel = (e_chunk >= lo) & (e_chunk < hi)
        ss = (e_chunk[sel] - lo) * SCS + e_slot[sel]
        srcg[ss] = src[sel]
        dstg[ss] = dst[sel]
        rel[ss] = e_rel[sel]
        first[ss] = e_first[sel].astype(np.float32)

        gsel = (chunk_of_group >= lo) & (chunk_of_group < hi)
        # rank-row index (node-sorted) of each of this core's nodes
        nodecr = ((chunk_of_group[gsel] - lo) * RK
                  + rank_of_group[gsel]).astype(np.int64)

        per_core.append(dict(
            nloc=nloc, dstbase=dstbase, srcg=srcg, dstg=dstg, nodecr=nodecr,
            rel=rel, first=first,
        ))

    return dict(per_core=per_core, scmax=scmax, n_sc=n_sc)


def host_weights(W1, att_src1, att_dst1, b1):
    """Pure re-layouts of weights (no data-sized arithmetic)."""
    W1 = np.asarray(W1, np.float32)
    W1T = np.ascontiguousarray(W1.T)                       # [D1, IC]
    Ablk = np.zeros((D1, 2 * H), np.float32)
    a_s = np.asarray(att_src1, np.float32)
    a_d = np.asarray(att_dst1, np.float32)
    for h in range(H):
        Ablk[h * F:(h + 1) * F, h] = a_s[h]
        Ablk[h * F:(h + 1) * F, H + h] = a_d[h]
    W1b = np.zeros((P, D1), np.float16)
    for h in range(H):
        W1b[9 + h * IC:9 + (h + 1) * IC, h * F:(h + 1) * F] = (
            W1[:, h * F:(h + 1) * F].astype(np.float16))
    W1b[8] = (np.asarray(b1, np.float32) + 1.0).astype(np.float16)
    # E9 [9, 121]: expands [1/s_h (8 rows) | 1] to per-xw-column scales
    E9 = np.zeros((9, TW), np.float16)
    for h in range(H):
        E9[h, 9 + h * IC:9 + (h + 1) * IC] = 1.0
    E9[8, 0:9] = 1.0
    return W1T, Ablk, W1b, E9


# ------------------------------------------------------------- launch 0 bass
def build_l0(npc):
    """Per-node asadT[16, npc] = ([W1@a_src | W1@a_dst].T) @ xT."""
    nt = npc // 512
    nc = bacc.Bacc("TRN2", target_bir_lowering=False)

    xt_in = nc.dram_tensor("xt", [IC, npc], FP16, kind="ExternalInput")
    w1t_in = nc.dram_tensor("w1t", [D1, IC], F32, kind="ExternalInput")
    ablk_in = nc.dram_tensor("ablk", [D1, 2 * H], F32, kind="ExternalInput")
    asadt_out = nc.dram_tensor("asadt", [2 * H, npc], FP16,
                               kind="ExternalOutput")

    with tile.TileContext(nc) as tc:
        with (
            tc.tile_pool(name="const", bufs=1) as cpool,
            tc.tile_pool(name="work", bufs=3) as wpool,
            tc.tile_pool(name="ps", bufs=4, space="PSUM") as ps,
        ):
            # fold AsAd[14, 16] = sum_b W1T_b.T @ Ablk_b
            w1t_t = wpool.tile([P, H, IC], F32, tag="w1tt")
            nc.sync.dma_start(
                out=w1t_t[:], in_=w1t_in.rearrange("(b p) k -> p b k", p=P))
            ablk_t = wpool.tile([P, H, 2 * H], F32, tag="ablkt")
            nc.sync.dma_start(
                out=ablk_t[:], in_=ablk_in.rearrange("(b p) k -> p b k", p=P))
            asad_ps = ps.tile([IC, 2 * H], F32, tag="fold", bufs=1)
            for b in range(H):
                nc.tensor.matmul(out=asad_ps[:], lhsT=w1t_t[:, b, :],
                                 rhs=ablk_t[:, b, :], start=(b == 0),
                                 stop=(b == H - 1))
            asad_h = cpool.tile([IC, 2 * H], FP16, tag="asadh")
            with nc.allow_low_precision(reason="fp16 logit weights"):
                nc.vector.tensor_copy(out=asad_h[:], in_=asad_ps[:])

            xt = cpool.tile([IC, npc], FP16, tag="xt")
            nc.sync.dma_start(out=xt[:], in_=xt_in[:, :])
            asb = cpool.tile([2 * H, npc], FP16, tag="asb")
            for t in range(nt):
                a_ps = ps.tile([2 * H, 512], F32, tag="aps", bufs=4)
                nc.tensor.matmul(out=a_ps[:], lhsT=asad_h[:],
                                 rhs=xt[:, t * 512:(t + 1) * 512],
                                 start=True, stop=True)
                with nc.allow_low_precision(reason="fp16 logits"):
                    if t % 2 == 0:
                        nc.scalar.copy(out=asb[:, t * 512:(t + 1) * 512],
                                       in_=a_ps[:])
                    else:
                        nc.vector.tensor_copy(
                            out=asb[:, t * 512:(t + 1) * 512], in_=a_ps[:])
            nc.sync.dma_start(out=asadt_out[:, :], in_=asb[:])

    nc.compile()
    return nc


# ------------------------------------------------------------- launch 1 bass
def build_l1(scmax):
    nb = scmax // SPB
    nc = bacc.Bacc("TRN2", target_bir_lowering=False)

    st_in = nc.dram_tensor("st", [P, scmax * SUBS, 32], FP16,
                           kind="ExternalInput")
    w1b_in = nc.dram_tensor("w1b", [P, D1], FP16, kind="ExternalInput")
    w2_in = nc.dram_tensor("w2", [D1, O2], F32, kind="ExternalInput")
    att2t_in = nc.dram_tensor("att2t", [O2, 2], F32, kind="ExternalInput")
    iota_in = nc.dram_tensor("iota", [P, W], FP16, kind="ExternalInput")
    identf_in = nc.dram_tensor("identf", [P, P], F32, kind="ExternalInput")
    e9_in = nc.dram_tensor("e9", [9, TW], FP16, kind="ExternalInput")
    t2rawt = nc.dram_tensor("t2rawt", [6, scmax * RK], F32,
                            kind="ExternalOutput")

    with tile.TileContext(nc) as tc:
        with (
            tc.tile_pool(name="const", bufs=1) as cpool,
            tc.tile_pool(name="work", bufs=3) as wpool,
            tc.tile_pool(name="chunk", bufs=4) as kpool,
            tc.tile_pool(name="ps", bufs=8, space="PSUM") as ps,
        ):
            identf = cpool.tile([P, P], F32, tag="identf")
            nc.sync.dma_start(out=identf[:], in_=identf_in[:, :])
            e9 = cpool.tile([9, TW], FP16, tag="e9")
            nc.sync.dma_start(out=e9[:], in_=e9_in[:, :])
            iota = cpool.tile([P, W], FP16, tag="iota")
            nc.sync.dma_start(out=iota[:], in_=iota_in[:, :])
            neg1 = cpool.tile([P, 1], F32, tag="neg1")
            nc.vector.memset(neg1[:], -1.0)
            ones = cpool.tile([P, 1], F32, tag="ones")
            nc.vector.memset(ones[:], 1.0)
            w1b = cpool.tile([P, D1], FP16, tag="w1b")
            nc.sync.dma_start(out=w1b[:], in_=w1b_in[:, :])

            # rhs6 [128, H, 6] = [va_b | vd_b | W2_b] fp16; -r6s [6,1]
            att2t_t = cpool.tile([O2, 2], F32, tag="att2t")
            nc.sync.dma_start(out=att2t_t[:], in_=att2t_in[:, :])
            rhs6f = wpool.tile([P, H, 6], F32, tag="rhs6f")
            for b in range(H):
                w2b = wpool.tile([P, O2], F32, tag="w2b")
                nc.sync.dma_start(out=w2b[:], in_=w2_in[b * P:(b + 1) * P, :])
                nc.vector.tensor_copy(out=rhs6f[:, b, 2:2 + O2], in_=w2b[:])
                w2bt_ps = ps.tile([O2, P], F32, tag="scr", bufs=1)
                nc.tensor.transpose(out=w2bt_ps[:], in_=w2b[:],
                                    identity=identf[:])
                w2bt = wpool.tile([O2, P], F32, tag="w2bt")
                nc.scalar.copy(out=w2bt[:], in_=w2bt_ps[:])
                vavd_ps = ps.tile([P, 2], F32, tag="scr", bufs=1)
                nc.tensor.matmul(out=vavd_ps[:], lhsT=w2bt[:], rhs=att2t_t[:],
                                 start=True, stop=True)
                nc.vector.tensor_copy(out=rhs6f[:, b, 0:2], in_=vavd_ps[:])
            rhs6 = cpool.tile([P, H, 6], FP16, tag="rhs6")
            with nc.allow_low_precision(reason="fp16 contraction weights"):
                nc.vector.tensor_copy(out=rhs6[:], in_=rhs6f[:])
            r6_ps = ps.tile([6, 1], F32, tag="acc", bufs=2)
            for b in range(H):
                nc.tensor.matmul(out=r6_ps[:], lhsT=rhs6f[:, b, :],
                                 rhs=ones[:], start=(b == 0), stop=(b == H - 1))
            negr6s = cpool.tile([6, 1], F32, tag="negr6s")
            nc.vector.tensor_scalar(out=negr6s[:], in0=r6_ps[:], scalar1=-1.0,
                                    scalar2=None, op0=mybir.AluOpType.mult)

            # whole slot table resident
            stall = cpool.tile([P, scmax * SUBS, 32], FP16, tag="stall")
            nc.sync.dma_start(out=stall[:], in_=st_in[:, :, :])

            # persistent full-width one-hot tiles; pad columns zeroed once,
            # only the fixed per-subchunk windows [Q*j, Q*j+W) are rewritten
            # each batch (single strided-AP is_equal per batch)
            oh_bufs = []
            for i in range(3):
                ohf = cpool.tile([P, G, P], FP16, tag=f"ohf{i}")
                nc.gpsimd.memset(ohf[:], 0.0)
                oh_bufs.append(ohf)
            rcp9_bufs = []
            for i in range(3):
                r9 = cpool.tile([9, SPB, P], FP16, tag=f"rcp9{i}")
                nc.vector.memset(r9[:], 1.0)
                rcp9_bufs.append(r9)

            def _winap(ohf):
                ap = ohf[:].copy()
                ap.ap = type(ap.ap)(
                    [[G * P, P], [SUBS * P, SPB], [P + Q, SUBS], [1, W]])
                return ap

            def batch(b):
                stb = stall[:, b * G:(b + 1) * G, :]
                z = kpool.tile([P, G, H], FP16, tag="z")
                nc.gpsimd.tensor_tensor(out=z[:], in0=stb[:, :, IC:IC + H],
                                        in1=stb[:, :, IC + H:IC + 2 * H],
                                        op=mybir.AluOpType.add)
                lg = kpool.tile([P, G, H], FP16, tag="lg")
                nc.vector.scalar_tensor_tensor(
                    out=lg[:], in0=z[:], scalar=NEG, in1=z[:],
                    op0=mybir.AluOpType.mult, op1=mybir.AluOpType.max)
                ew = kpool.tile([P, G, H], FP16, tag="ew")
                nc.scalar.activation(ew[:], lg[:],
                                     mybir.ActivationFunctionType.Exp)
                ohf = oh_bufs[b % 3]
                nc.vector.tensor_tensor(
                    out=_winap(ohf),
                    in0=iota[:].rearrange("p (a c r) -> p a c r", a=1, c=1)
                        .to_broadcast([P, SPB, SUBS, W]),
                    in1=stb[:, :, 30:31].rearrange(
                        "p (s j) k -> p s j k", j=SUBS)
                        .to_broadcast([P, SPB, SUBS, W]),
                    op=mybir.AluOpType.is_equal)
                xw = kpool.tile([P, G, TW], FP16, tag="xw")
                for s_ in range(SPB):
                    c0 = s_ * SUBS
                    nc.gpsimd.tensor_tensor(
                        out=xw[:, c0:c0 + SUBS, 9:TW].rearrange(
                            "p g (h k) -> p g h k", h=H),
                        in0=stb[:, c0:c0 + SUBS, 0:IC].rearrange(
                            "p g (a k) -> p g a k", a=1)
                            .to_broadcast([P, SUBS, H, IC]),
                        in1=ew[:, c0:c0 + SUBS, :].rearrange(
                            "p g (h a) -> p g h a", a=1)
                            .to_broadcast([P, SUBS, H, IC]),
                        op=mybir.AluOpType.mult)
                nc.vector.tensor_copy(out=xw[:, :, 0:H], in_=ew[:])
                nc.vector.tensor_copy(out=xw[:, :, H:H + 1],
                                      in_=stb[:, :, 31:32])

                # flipped scatter: t2T[xw-col, rank] accumulates per sc
                t2_ps = ps.tile([TW, SPB, P], F32, tag="t2ps", bufs=2)
                for s in range(SPB):
                    for j in range(SUBS):
                        c = s * SUBS + j
                        nc.tensor.matmul(out=t2_ps[:, s, :],
                                         lhsT=xw[:, c, :],
                                         rhs=ohf[:, c, :],
                                         start=(j == 0),
                                         stop=(j == SUBS - 1))
                # node phase, transposed: rcp9 rows 0:8 = 1/s, row 8 = 1
                r32 = wpool.tile([8, SPB, P], F32, tag="r32")
                nc.vector.reciprocal_approx_fast(out=r32[:],
                                                 in_=t2_ps[0:8, :, :])
                rcp9 = rcp9_bufs[b % 3]
                with nc.allow_low_precision(reason="fp16 1/s"):
                    nc.vector.tensor_copy(out=rcp9[0:8, :, :], in_=r32[:])
                rc_ps = ps.tile([TW, SPB, P], F32, tag="scr", bufs=1)
                nc.tensor.matmul(out=rc_ps[:, :, :], lhsT=e9[:],
                                 rhs=rcp9[:, :, :], start=True, stop=True)
                rcs = wpool.tile([TW, SPB, P], FP16, tag="rcs")
                with nc.allow_low_precision(reason="fp16 scales"):
                    nc.scalar.copy(out=rcs[:], in_=rc_ps[:])
                tnt = kpool.tile([TW, SPB, P], FP16, tag="tnt")
                nc.vector.tensor_tensor(out=tnt[:], in0=t2_ps[:],
                                        in1=rcs[:],
                                        op=mybir.AluOpType.mult)

                h2_ps = ps.tile([6, BS], F32, tag="acc", bufs=2)
                for h in range(H):
                    o1_ps = ps.tile([P, BS], F32, tag="o1ps", bufs=3)
                    nc.tensor.matmul(out=o1_ps[:],
                                     lhsT=w1b[0:TW, h * F:(h + 1) * F],
                                     rhs=tnt[:, :, :], start=True, stop=True)
                    e = wpool.tile([P, BS], FP16, tag="e")
                    nc.scalar.activation(e[:], o1_ps[:],
                                         mybir.ActivationFunctionType.Exp,
                                         bias=neg1[:, 0:1])
                    e1 = wpool.tile([P, BS], FP16, tag="e1")
                    nc.vector.scalar_tensor_tensor(
                        out=e1[:], in0=e[:], scalar=1.0, in1=o1_ps[:],
                        op0=mybir.AluOpType.min, op1=mybir.AluOpType.max)
                    nc.tensor.matmul(out=h2_ps[:], lhsT=rhs6[:, h, :],
                                     rhs=e1[:], start=(h == 0),
                                     stop=(h == H - 1))
                h2sb = wpool.tile([6, BS], F32, tag="h2sb")
                nc.scalar.activation(h2sb[:], h2_ps[:],
                                     mybir.ActivationFunctionType.Identity,
                                     bias=negr6s[:, 0:1])
                nc.sync.dma_start(out=t2rawt[:, b * BS:(b + 1) * BS],
                                  in_=h2sb[:])

            for b in range(nb):
                batch(b)

    nc.compile()
    return nc


# ------------------------------------------------------------- launch 2 bass
def build_l2(scmax):
    nb = scmax // SPB
    nc = bacc.Bacc("TRN2", target_bir_lowering=False)

    st_in = nc.dram_tensor("st6", [P, scmax * SUBS, 8], FP16,
                           kind="ExternalInput")
    iota_in = nc.dram_tensor("iota", [P, W], FP16, kind="ExternalInput")
    b2t_in = nc.dram_tensor("b2t", [O2, 1], F32, kind="ExternalInput")
    identf_in = nc.dram_tensor("identf", [P, P], F32, kind="ExternalInput")
    out2 = nc.dram_tensor("out2", [P, scmax // SPB, SPB, O2], F32,
                          kind="ExternalOutput")

    with tile.TileContext(nc) as tc:
        with (
            tc.tile_pool(name="const", bufs=1) as cpool,
            tc.tile_pool(name="chunk", bufs=4) as kpool,
            tc.tile_pool(name="ps", bufs=8, space="PSUM") as ps,
        ):
            identf = cpool.tile([P, P], F32, tag="identf")
            nc.sync.dma_start(out=identf[:], in_=identf_in[:, :])
            b2t_t = cpool.tile([O2, 1], F32, tag="b2t")
            nc.sync.dma_start(out=b2t_t[:], in_=b2t_in[:, :])
            b2_ps = ps.tile([P, O2], F32, tag="setup", bufs=1)
            nc.tensor.transpose(out=b2_ps[:],
                                in_=b2t_t[:].to_broadcast([O2, P]),
                                identity=identf[0:O2, 0:O2])
            b2bc = cpool.tile([P, O2], F32, tag="b2bc")
            nc.vector.tensor_copy(out=b2bc[:], in_=b2_ps[:])
            iota = cpool.tile([P, W], FP16, tag="iota")
            nc.sync.dma_start(out=iota[:], in_=iota_in[:, :])

            stall = cpool.tile([P, scmax * SUBS, 8], FP16, tag="stall")
            nc.sync.dma_start(out=stall[:], in_=st_in[:, :, :])
            outsb = cpool.tile([P, nb, SPB, O2], F32, tag="outsb")

            oh_bufs = []
            for i in range(3):
                ohf = cpool.tile([P, G, P], FP16, tag=f"ohf{i}")
                nc.gpsimd.memset(ohf[:], 0.0)
                oh_bufs.append(ohf)
            rcp9_bufs = []
            for i in range(3):
                r9 = cpool.tile([9, SPB, P], FP16, tag=f"rcp9{i}")
                nc.vector.memset(r9[:], 1.0)
                rcp9_bufs.append(r9)

            def _winap(ohf):
                ap = ohf[:].copy()
                ap.ap = type(ap.ap)(
                    [[G * P, P], [SUBS * P, SPB], [P + Q, SUBS], [1, W]])
                return ap

            def batch(b):
                stb = stall[:, b * G:(b + 1) * G, :]
                z = kpool.tile([P, G, 1], FP16, tag="z")
                nc.gpsimd.tensor_tensor(out=z[:], in0=stb[:, :, 0:1],
                                        in1=stb[:, :, 1:2],
                                        op=mybir.AluOpType.add)
                lg = kpool.tile([P, G, 1], FP16, tag="lg")
                nc.vector.scalar_tensor_tensor(
                    out=lg[:], in0=z[:], scalar=NEG, in1=z[:],
                    op0=mybir.AluOpType.mult, op1=mybir.AluOpType.max)
                ew = kpool.tile([P, G, 1], FP16, tag="ew")
                nc.scalar.activation(ew[:], lg[:],
                                     mybir.ActivationFunctionType.Exp)
                r5 = kpool.tile([P, G, O2 + 1], FP16, tag="r5")
                nc.gpsimd.tensor_tensor(
                    out=r5[:, :, 0:O2], in0=stb[:, :, 2:6],
                    in1=ew[:].to_broadcast([P, G, O2]),
                    op=mybir.AluOpType.mult)
                nc.vector.tensor_copy(out=r5[:, :, O2:O2 + 1], in_=ew[:])

                ohf = oh_bufs[b % 3]
                nc.vector.tensor_tensor(
                    out=_winap(ohf),
                    in0=iota[:].rearrange("p (a c r) -> p a c r", a=1, c=1)
                        .to_broadcast([P, SPB, SUBS, W]),
                    in1=stb[:, :, 6:7].rearrange(
                        "p (s j) k -> p s j k", j=SUBS)
                        .to_broadcast([P, SPB, SUBS, W]),
                    op=mybir.AluOpType.is_equal)
                o5_ps = ps.tile([P, SPB, O2 + 1], F32, tag="o5ps", bufs=3)
                for s in range(SPB):
                    for j in range(SUBS):
                        c = s * SUBS + j
                        nc.tensor.matmul(out=o5_ps[:, s, :],
                                         lhsT=ohf[:, c, :],
                                         rhs=r5[:, c, :],
                                         start=(j == 0),
                                         stop=(j == SUBS - 1))
                rcp = kpool.tile([P, SPB, 1], F32, tag="rcp")
                nc.vector.reciprocal(out=rcp[:], in_=o5_ps[:, :, O2:O2 + 1])
                of = kpool.tile([P, SPB, O2], F32, tag="of")
                nc.vector.tensor_tensor(out=of[:], in0=o5_ps[:, :, 0:O2],
                                        in1=rcp[:].to_broadcast(
                                            [P, SPB, O2]),
                                        op=mybir.AluOpType.mult)
                nc.gpsimd.tensor_tensor(
                    out=outsb[:, b, :, :], in0=of[:],
                    in1=b2bc[:].rearrange("p (a k) -> p a k", a=1)
                        .to_broadcast([P, SPB, O2]),
                    op=mybir.AluOpType.add)

            for b in range(nb):
                batch(b)
            nc.sync.dma_start(out=out2[:, :, :, :], in_=outsb[:])

    nc.compile()
    return nc


# ------------------------------------------------------------------- kernel
_cache = {}
_pack_cache = {}
LAST_EXEC_NS = []  # [l0, l1, l2] when GAT_TRACE=1


def kernel(x, edge_index, W1, att_src1, att_dst1, b1, W2, att_src2,
           att_dst2, b2):
    x = np.asarray(x, np.float32)
    n_nodes = x.shape[0]
    ei = np.asarray(edge_index)

    pkey = (hash(ei.tobytes()), n_nodes)
    if pkey not in _pack_cache:
        _pack_cache[pkey] = pack_graph(ei, n_nodes)
    pk = _pack_cache[pkey]
    scmax = pk["scmax"]

    npc0 = -(-max(-(-n_nodes // N_CORES), 1) // 512) * 512

    key = (n_nodes, scmax, npc0)
    if key not in _cache:
        _cache[key] = (build_l0(npc0), build_l1(scmax), build_l2(scmax))
    nc0, nc1, nc2 = _cache[key]

    W1T, Ablk, W1b, E9 = host_weights(W1, att_src1, att_dst1, b1)
    att2T = np.stack([np.asarray(att_src2, np.float32)[0],
                      np.asarray(att_dst2, np.float32)[0]], axis=1)  # [4, 2]
    iota = np.tile(np.arange(W, dtype=np.float16)[None, :], (P, 1))
    identf = np.eye(P, dtype=np.float32)

    x16 = x.astype(np.float16)

    if _trace:
        for sfx in ("_l0", "_l1", "_l2"):
            shutil.rmtree(_trace_dir + sfx, ignore_errors=True)
    LAST_EXEC_NS.clear()

    # ---- L0: per-node attention logits
    npern = -(-n_nodes // N_CORES)
    in_maps0 = []
    for k in range(N_CORES):
        lo = k * npern
        hi = min(lo + npern, n_nodes)
        xt = np.zeros((IC, npc0), np.float16)
        xt[:, 0:hi - lo] = x16[lo:hi].T
        in_maps0.append({"xt": xt, "w1t": W1T, "ablk": Ablk})
    res0 = run_bass_kernel_spmd(
        nc0, in_maps0, list(range(N_CORES)), trace=_trace,
        tmpdir=(_trace_dir + "_l0") if _trace else None)
    if _trace:
        print("L0 exec_time_ns:", res0.exec_time_ns)
        LAST_EXEC_NS.append(res0.exec_time_ns)
    asad = np.empty((n_nodes, 2 * H), np.float16)
    for k in range(N_CORES):
        lo = k * npern
        hi = min(lo + npern, n_nodes)
        asad[lo:hi] = res0.results[k]["asadt"][:, 0:hi - lo].T

    # ---- L1: edge pass 1 + node pass
    in_maps1 = []
    for k in range(N_CORES):
        pc = pk["per_core"][k]
        sflat = pc["srcg"]
        dflat = pc["dstg"]
        st = np.empty((scmax * SCS, 32), np.float16)
        st[:, 0:IC] = x16[sflat]
        st[:, IC:IC + H] = asad[sflat, 0:H]
        st[:, IC + H:IC + 2 * H] = asad[dflat, H:2 * H]
        st[:, 30] = pc["rel"]
        st[:, 31] = pc["first"]
        st = np.ascontiguousarray(
            st.reshape(scmax * SUBS, P, 32).transpose(1, 0, 2))
        in_maps1.append({
            "st": st, "w1b": W1b, "w2": np.asarray(W2, np.float32),
            "att2t": att2T, "iota": iota, "identf": identf, "e9": E9,
        })
    res1 = run_bass_kernel_spmd(
        nc1, in_maps1, list(range(N_CORES)), trace=_trace,
        tmpdir=(_trace_dir + "_l1") if _trace else None)
    if _trace:
        print("L1 exec_time_ns:", res1.exec_time_ns)
        LAST_EXEC_NS.append(res1.exec_time_ns)

    # host assembly: rank rows -> node rows
    h2full = np.empty((n_nodes, 6), np.float32)
    for k in range(N_CORES):
        pc = pk["per_core"][k]
        h2full[pc["dstbase"]:pc["dstbase"] + pc["nloc"]] = (
            res1.results[k]["t2rawt"][:, pc["nodecr"]].T)
    h2f16 = h2full.astype(np.float16)

    # ---- L2: edge pass 2
    in_maps2 = []
    for k in range(N_CORES):
        pc = pk["per_core"][k]
        sflat = pc["srcg"]
        dflat = pc["dstg"]
        st6 = np.zeros((scmax * SCS, 8), np.float16)
        st6[:, 0] = h2f16[sflat, 0]              # as2[src]
        st6[:, 1] = h2f16[dflat, 1]              # ad2[dst]
        st6[:, 2:6] = h2f16[sflat, 2:6]          # h2[src]
        st6[:, 6] = pc["rel"]
        st6 = np.ascontiguousarray(
            st6.reshape(scmax * SUBS, P, 8).transpose(1, 0, 2))
        in_maps2.append({
            "st6": st6, "iota": iota, "identf": identf,
            "b2t": np.asarray(b2, np.float32).reshape(O2, 1),
        })
    res2 = run_bass_kernel_spmd(
        nc2, in_maps2, list(range(N_CORES)), trace=_trace,
        tmpdir=(_trace_dir + "_l2") if _trace else None)
    if _trace:
        print("L2 exec_time_ns:", res2.exec_time_ns)
        LAST_EXEC_NS.append(res2.exec_time_ns)

    out = np.zeros((n_nodes, O2), np.float32)
    for k in range(N_CORES):
        pc = pk["per_core"][k]
        o = res2.results[k]["out2"].transpose(1, 2, 0, 3).reshape(-1, O2)
        out[pc["dstbase"]:pc["dstbase"] + pc["nloc"]] = o[pc["nodecr"]]
    return out
